# revision 15
# baseline (speedup 1.0000x reference)
"""Trainium2 Bass kernel for BaseMultiheadAttention.

dims: B=1, V=4, S=2048, E=512, H=8, D=64 (head_dim), causal, interleaved RoPE.

Sharding (8 cores): core c -> bv index g = c//2, head-group hg = c%2
(4 heads each).  Each core computes its bv-slice's QKV projection restricted
to its 4 heads, RoPE, causal attention, and a partial output projection
(its heads' wO rows).  Host sums the two partials per bv index.

v2 design notes (per core, fp32 storage, matmuls in float32r):
  x        arrives UNTRANSPOSED (2048, 512); PE transposes 128x128 blocks
           (is_transpose matmul vs identity) into xt_all (128, 4*2048)
           [e-chunk-major], drained psum->SBUF by ACT copies.
  qT/kT    two 128-partition stacks (2 heads each), d on partitions, via
           matmul (w chunk stationary, xt moving).  RoPE = DVE shuffle +
           2 mul + add with host-precomputed cos / sign-folded sin tables.
  v        vt_all (128, 16*260): per k-tile, 4 heads of (64 cols + ones
           col) -> PV accumulates softmax denominator in psum row 64.
  scores   S^T (s_k on part, s_q free) per (hp, j, i) into a (128,1024)
           psum pair (2 heads); exp on ACT (scale=1/8); causal diag handled
           AFTER exp by a DVE multiply with a 0/1 upper-tri mask (128 cols
           per diag tile) -- no bias-prefill matmuls.
  pipeline the attention loop is software-pipelined: PV for iteration i-1
           is emitted after scores/exp of iteration i, so PE never waits
           on ACT.  Block tails (final PV + normalize) are deferred into
           the next block; outproj chunks are drip-fed between iterations.
  norm     denom row -> DVE reciprocal (1,512) -> gpsimd partition
           broadcast -> DVE multiply into onorm (o^T/denom).
  outproj  (s,e)-native: onorm s-chunk stationary, woT moving, psum
           (128 s, 512 e) -> DVE copy -> DMA straight to out rows.
"""

import numpy as np

import concourse.bass as bass
import concourse.mybir as mybir
from concourse.tile import TileContext
from concourse import library_config
from concourse.bass_utils import run_bass_kernel_spmd

# ---- problem dims (hardcoded per the task contract) ----
B, V, S, E, H = 1, 4, 2048, 512, 8
D = E // H            # 64
HG = 4                # heads per core
NCORE = 8
NT = S // 128         # 16 s_k tiles
FR = mybir.dt.float32r
F32 = mybir.dt.float32


def _host_tables():
    pos = np.arange(S, dtype=np.float64)
    inv_freq = 1.0 / (10000.0 ** (np.arange(0, D, 2, dtype=np.float64) / D))
    freqs = pos[:, None] * inv_freq[None, :]          # (S, D/2)
    freqs = np.repeat(freqs, 2, axis=-1)              # (S, D) interleaved
    cosT = np.cos(freqs).T.astype(np.float32)         # (D, S)
    sinT = np.sin(freqs).T.astype(np.float32)
    cs = np.concatenate([cosT, cosT], axis=0)         # (128, S) two-head stack
    sn = np.concatenate([sinT, sinT], axis=0)
    # upper-tri keep mask for a diag tile: keep col >= p, duplicated for the
    # two heads of a stack so one strided DVE multiply covers both.
    tri = np.zeros((128, 128), dtype=np.float32)
    for p in range(128):
        tri[p, p:] = 1.0
    tri01d = np.tile(tri, (1, 2))                     # (128, 256)
    iden = np.eye(128, dtype=np.float32)
    return cs, sn, tri01d, iden


def _host_weights(wqkv_w, wqkv_b, wo_w, hg):
    """Per-head-group weight slices in the kernel's layouts."""
    heads = [hg * HG + h for h in range(HG)]
    cs, sn, tri01d, iden = _host_tables()
    # feature index inside each qkv block: d*H + h  (d fastest-major: index = d*8+h)
    def rows(block, h):
        d = np.arange(D)
        return block * E + d * H + h
    Wq = np.stack([wqkv_w[rows(0, h)] for h in heads])   # (HG, D, E)
    Wk = np.stack([wqkv_w[rows(1, h)] for h in heads])
    Wv = np.stack([wqkv_w[rows(2, h)] for h in heads])
    def to_T(Wh):   # (HG, D, E) -> (E, HG*D) with col = h*64+d
        return np.ascontiguousarray(
            np.transpose(Wh, (2, 0, 1)).reshape(E, HG * D)).astype(np.float32)
    wqT = to_T(Wq)
    wkT = to_T(Wk)
    wvT = to_T(Wv)
    # wo rows for this head group: out feature = h_global*64 + d
    woT = np.stack([wo_w[:, (hg * HG + h) * D:(hg * HG + h + 1) * D].T
                    for h in range(HG)])                 # (HG, D, E)
    woT = np.ascontiguousarray(woT.reshape(HG * D, E)).astype(np.float32)
    sgn = np.tile(np.array([-1.0, 1.0], np.float32), D // 2)[:, None]
    snS = sn * np.concatenate([sgn, sgn], axis=0)
    return dict(wqT=wqT, wkT=wkT, wvT=wvT, woT=woT,
                cs=cs, snS=snS, tri01d=tri01d, iden=iden,
                vones=np.ones((128, NT * HG), np.float32),
                ones64=np.ones((1, D), np.float32))


_MAX_WAITS = {"Matmult": 1}          # per-opcode cap; default below
_DEF_MAX_WAITS = 1


def _split_excess_waits(nc):
    """This walrus build encodes at most ~1 sync-wait per instruction.
    Post-process the serialized BIR: hoist excess on_wait entries onto
    same-engine NoOp carriers emitted immediately before the instruction."""
    import orjson

    orig = nc.to_json_bytes

    def patched(_self=None):
        d = orjson.loads(orig())
        for fn in d.get("functions", []):
            for bb in fn.get("basicblocks", fn.get("blocks", [])):
                insts = bb.get("instructions")
                if insts is None:
                    continue
                out, nctr = [], 0
                for inst in insts:
                    si = inst.get("sync_info")
                    waits = (si or {}).get("on_wait") or []
                    cap = _MAX_WAITS.get(inst.get("opcode"), _DEF_MAX_WAITS)
                    if len(waits) > cap:
                        keep = waits[:cap]
                        extra = waits[cap:]
                        for w in extra:
                            nctr += 1
                            out.append({
                                "debug": inst.get("debug", 0),
                                "engine": inst["engine"],
                                "ins": [], "outs": [],
                                "name": f"{inst['name']}_w{nctr}",
                                "opcode": "NoOp",
                                "sync_info": {"on_wait": [w],
                                              "on_update": []},
                            })
                        si["on_wait"] = keep
                    out.append(inst)
                bb["instructions"] = out
        return orjson.dumps(d)

    nc.to_json_bytes = patched
    return nc


def build_nc(loop_n=0):
    nc = bass.Bass()
    x = nc.declare_dram_parameter("x", [S, E], FR, isOutput=False)
    wqT = nc.declare_dram_parameter("wqT", [E, HG * D], FR, isOutput=False)
    wkT = nc.declare_dram_parameter("wkT", [E, HG * D], FR, isOutput=False)
    wvT = nc.declare_dram_parameter("wvT", [E, HG * D], FR, isOutput=False)
    woT = nc.declare_dram_parameter("woT", [HG * D, E], FR, isOutput=False)
    cs = nc.declare_dram_parameter("cs", [128, S], F32, isOutput=False)
    snS = nc.declare_dram_parameter("snS", [128, S], F32, isOutput=False)
    tri01d = nc.declare_dram_parameter("tri01d", [128, 256], F32,
                                       isOutput=False)
    iden = nc.declare_dram_parameter("iden", [128, 128], FR, isOutput=False)
    vones = nc.declare_dram_parameter("vones", [128, NT * HG], F32,
                                      isOutput=False)
    ones64 = nc.declare_dram_parameter("ones64", [1, D], FR, isOutput=False)
    out = nc.declare_dram_parameter("out", [S, E], F32, isOutput=True)

    fr = lambda ap: ap.bitcast(FR)
    SWAP = [1, 0, 3, 2, 5, 4, 7, 6, 9, 8, 11, 10, 13, 12, 15, 14,
            17, 16, 19, 18, 21, 20, 23, 22, 25, 24, 27, 26, 29, 28, 31, 30]
    scale = 1.0 / np.sqrt(D)

    from contextlib import ExitStack
    with TileContext(nc) as tc:
      with ExitStack() as _lp:
        if loop_n:
            _lp.enter_context(tc.For_i(0, loop_n))
        with (
            tc.tile_pool(name="const", bufs=1) as cpool,
            tc.tile_pool(name="qk", bufs=1) as qkpool,
            tc.tile_pool(name="pt", bufs=2) as ptpool,
            tc.tile_pool(name="on", bufs=1) as onpool,
            tc.tile_pool(name="sums", bufs=2) as spool,
            tc.tile_pool(name="oc", bufs=3) as ocpool,
            tc.tile_pool(name="psA", bufs=1, space="PSUM") as psA,
        ):
            # ---------- constant / weight loads ----------
            # scalar (ACT) queue: iden first (needed by the transposes),
            # then wv (vproj), wq/wk, wo, rope tables, tri mask.
            iden_t = cpool.tile([128, 128], FR, tag="iden", name="iden_t")
            nc.scalar.dma_start(iden_t[:, :], iden[:, :])
            wv_t, wq_t, wk_t = [], [], []
            for nm, dram, lst in (("wv", wvT, wv_t), ("wq", wqT, wq_t),
                                  ("wk", wkT, wk_t)):
                for e4 in range(4):
                    t = cpool.tile([128, HG * D], FR, tag=f"{nm}{e4}",
                                   name=f"{nm}{e4}")
                    nc.scalar.dma_start(t[:, :], dram[e4 * 128:(e4 + 1) * 128, :])
                    lst.append(t)
            wo_t = []
            for f2 in range(2):
                t = cpool.tile([128, E], FR, tag=f"wo{f2}", name=f"wo{f2}")
                nc.scalar.dma_start(t[:, :], woT[f2 * 128:(f2 + 1) * 128, :])
                wo_t.append(t)
            cs_t = cpool.tile([128, S], F32, tag="cs", name="cs_t")
            sn_t = cpool.tile([128, S], F32, tag="sn", name="sn_t")
            nc.scalar.dma_start(cs_t[:, :], cs[:, :])
            nc.scalar.dma_start(sn_t[:, :], snS[:, :])
            tri_t = cpool.tile([128, 256], F32, tag="tri", name="tri_t")
            nc.scalar.dma_start(tri_t[:, :], tri01d[:, :])
            ones_t = cpool.tile([1, D], FR, tag="ones64", name="ones_t")
            nc.scalar.dma_start(ones_t[:, :], ones64[:, :])

            # ---------- x load + PE transpose + V projection ----------
            # sync queue: x tiles (contiguous row blocks)
            xs = []

            def emit_xs_dma(i):
                t = cpool.tile([128, E], FR, tag="xs", bufs=3, name=f"xs{i}")
                nc.sync.dma_start(t[:, :], x[i * 128:(i + 1) * 128, :])
                xs.append(t)
            # xt_all: x^T, e-chunk-major: chunk e4 at cols [e4*S, (e4+1)*S)
            xt_all = cpool.tile([128, 4 * S], FR, tag="xt", name="xt_all")
            xtc = lambda e4, c0, w: xt_all[:, e4 * S + c0:e4 * S + c0 + w]
            # vt_all: 16 k-tiles of (4 heads x 65) with ones col at offset 64
            vt_all = cpool.tile([128, NT * HG * 65], FR, tag="vt",
                                name="vt_all")
            nc.scalar.dma_start(
                vt_all[:, :].rearrange("p (t h x) -> p t h x",
                                       t=NT, h=HG)[:, :, :, D:D + 1],
                vones[:, :, None].bitcast(FR))

            def emit_transpose(i):
                tp = psA.tile([128, 512], F32, tag="b1", bufs=2,
                              name=f"tp{i}")
                for e4 in range(4):
                    nc.tensor.transpose(
                        fr(tp[:, e4 * 128:(e4 + 1) * 128]),
                        xs[i][:, e4 * 128:(e4 + 1) * 128],
                        iden_t[:, :])
                nc.scalar.copy(
                    xt_all[:, :].rearrange("p (e s) -> p e s",
                                           e=4)[:, :, i * 128:(i + 1) * 128],
                    tp[:, :].rearrange("p (e s) -> p e s", e=4))

            def emit_vproj(i):
                pv = psA.tile([128, 512], F32, tag="b1", bufs=2,
                              name=f"pv{i}")
                for e4 in range(4):
                    nc.tensor.matmul(
                        pv[:, 0:HG * D],
                        xtc(e4, i * 128, 128),
                        wv_t[e4][:, :], start=(e4 == 0), stop=(e4 == 3))
                nc.scalar.copy(
                    vt_all[:, i * HG * 65:(i + 1) * HG * 65].rearrange(
                        "p (h x) -> p h x", h=HG)[:, :, 0:D],
                    pv[:, 0:HG * D].rearrange("p (h d) -> p h d", h=HG))

            for i in range(3):
                emit_xs_dma(i)
            for i in range(NT):
                if i + 3 < NT:
                    emit_xs_dma(i + 3)
                emit_transpose(i)
                if i >= 1:
                    emit_vproj(i - 1)
            emit_vproj(NT - 1)

            # ---------- q/k projections + RoPE ----------
            qrot, krot = {}, {}

            def project_rope(tgt, wlist, dst, st):
                rt = qkpool.tile([128, S], FR, tag=f"{tgt}rot{st}",
                                 name=f"{tgt}rot{st}")
                dst[st] = rt
                fcol = st * 2 * D
                for half in range(2):
                    s0 = half * 1024
                    pq = psA.tile([128, 1024], F32, tag="duo",
                                  bufs=2, name="pq")
                    for nb in range(2):
                        c0 = s0 + nb * 512
                        for e4 in range(4):
                            nc.tensor.matmul(
                                pq[:, nb * 512:(nb + 1) * 512],
                                wlist[e4][:, fcol:fcol + 128],
                                xtc(e4, c0, 512),
                                start=(e4 == 0), stop=(e4 == 3))
                    t1 = ptpool.tile([128, 1024], F32, tag="ropetmp",
                                     name="ropetmp")
                    sh_t = ptpool.tile([128, 1024], F32, tag="ropesh",
                                       name="ropesh")
                    nc.vector.stream_shuffle(sh_t[:, :], pq[:, :], SWAP)
                    nc.vector.tensor_mul(
                        t1[:, :], pq[:, :], cs_t[:, s0:s0 + 1024])
                    nc.vector.tensor_mul(
                        sh_t[:, :], sh_t[:, :], sn_t[:, s0:s0 + 1024])
                    nc.vector.tensor_add(
                        rt[:, s0:s0 + 1024], sh_t[:, :], t1[:, :])

            project_rope("k", wk_t, krot, 0)
            project_rope("q", wq_t, qrot, 0)
            project_rope("k", wk_t, krot, 1)
            project_rope("q", wq_t, qrot, 1)

            # ---------- attention, software-pipelined ----------
            onorm = {}

            def emit_scores(hp, j, i, po):
                """Scores + exp (+ diag mask) for k-tile i; returns the pt
                tile and a callback that emits the two PV matmuls."""
                r = i - 4 * j
                offs = 128 * r if r >= 0 else 0
                sc = psA.tile([128, 1024], F32, tag="duo", bufs=2, name="sc")
                for hh in range(2):
                    nc.tensor.matmul(
                        sc[:, hh * 512 + offs:hh * 512 + 512],
                        krot[hp][hh * D:hh * D + D,
                                 i * 128:(i + 1) * 128],
                        qrot[hp][hh * D:hh * D + D,
                                 j * 512 + offs:(j + 1) * 512],
                        start=True, stop=True,
                        tile_position=(hh * D, 0))
                pt = ptpool.tile([128, 1024], FR, tag="pt", bufs=3,
                                 name="pt")
                if offs == 0:
                    nc.scalar.activation(
                        pt[:, :], sc[:, :],
                        mybir.ActivationFunctionType.Exp, scale=float(scale))
                else:
                    nc.scalar.activation(
                        pt[:, :].rearrange("p (h x) -> p h x",
                                           h=2)[:, :, offs:512],
                        sc[:, :].rearrange("p (h x) -> p h x",
                                           h=2)[:, :, offs:512],
                        mybir.ActivationFunctionType.Exp, scale=float(scale))
                if r >= 0:
                    ptv = pt[:, :].rearrange("p (h x) -> p h x",
                                             h=2)[:, :, offs:offs + 128]
                    nc.vector.tensor_mul(
                        ptv, ptv,
                        tri_t[:, :].rearrange("p (h x) -> p h x", h=2))

                def emit_pv():
                    for hh in range(2):
                        h = 2 * hp + hh
                        nc.tensor.matmul(
                            po[hh][0:65, offs:512],
                            vt_all[:, i * HG * 65 + h * 65:
                                   i * HG * 65 + h * 65 + 65],
                            pt[:, hh * 512 + offs:hh * 512 + 512],
                            start=(i == 0), stop=(i == 4 * j + 3))
                return emit_pv

            def make_tail(hp, j, po, last_pv, opq):
                def tail():
                    last_pv()
                    onj = onpool.tile([128, 512], FR, tag=f"on{j}{hp}",
                                      name=f"on{j}{hp}")
                    onorm[(j, hp)] = onj
                    for hh in range(2):
                        srow = spool.tile([1, 512], FR, tag="srow",
                                          name="srow")
                        rb = spool.tile([64, 512], F32, tag="rb", name="rb")
                        nc.vector.tensor_copy(srow[:, :], po[hh][64:65, :])
                        bc = psA.tile([128, 512], F32, tag="b1", bufs=2,
                                      name="bc")
                        nc.tensor.matmul(
                            bc[0:D, :], ones_t[:, :], srow[:, :],
                            start=True, stop=True)
                        nc.vector.reciprocal(rb[:, :], bc[0:D, :])
                        nc.vector.tensor_mul(
                            onj[hh * D:hh * D + D, :],
                            po[hh][0:D, :], rb[:, :])
                    if hp == 1:
                        for sc4 in range(4):
                            opq.append(make_outproj(j, sc4))
                return tail

            def make_outproj(j, sc4):
                def emit():
                    pp = psA.tile([128, 512], F32, tag="b1", bufs=2,
                                  name="pp")
                    for f2 in range(2):
                        nc.tensor.matmul(
                            pp[:, :],
                            onorm[(j, f2)][:, sc4 * 128:(sc4 + 1) * 128],
                            wo_t[f2][:, :],
                            start=(f2 == 0), stop=(f2 == 1))
                    oc = ocpool.tile([128, 512], F32, tag="oc", name="oc")
                    nc.vector.tensor_copy(oc[:, :], pp[:, :])
                    nc.sync.dma_start(
                        out[j * 512 + sc4 * 128:j * 512 + (sc4 + 1) * 128, :],
                        oc[:, :])
                return emit

            blocks = [(0, j) for j in range(4)] + [(1, j) for j in range(4)]
            prev_tail = None
            opq = []
            for hp, j in blocks:
                po = [psA.tile([128, 512], F32, tag="po", bufs=2,
                               name=f"po{hh}") for hh in range(2)]
                carried = None
                for i in range(4 * j + 4):
                    nxt = emit_scores(hp, j, i, po)
                    if i == 0:
                        if prev_tail is not None:
                            prev_tail()
                    else:
                        carried()
                        if opq and i >= 2:
                            opq.pop(0)()
                    carried = nxt
                prev_tail = make_tail(hp, j, po, carried, opq)
            prev_tail()
            while opq:
                opq.pop(0)()
    return _split_excess_waits(nc)


_NC_CACHE = {}


def _get_nc(loop_n=0):
    if loop_n not in _NC_CACHE:
        _NC_CACHE[loop_n] = build_nc(loop_n=loop_n)
    return _NC_CACHE[loop_n]


_RUNNER_CACHE = {}


def _get_runner(nc, n_cores):
    """bass2jax multi-core path with the jitted callable cached, constant
    inputs device-committed once, and output zero-buffers created on
    device (nothing but x is shipped per call)."""
    key = id(nc)
    if key in _RUNNER_CACHE:
        return _RUNNER_CACHE[key]
    import jax
    import jax.numpy as jnp
    from jax.sharding import Mesh, PartitionSpec, NamedSharding
    from jax.experimental.shard_map import shard_map
    from concourse import bass2jax as b2j

    b2j.install_neuronx_cc_hook()
    partition_name = (nc.partition_id_tensor.name
                      if nc.partition_id_tensor else None)
    in_names, out_names, out_avals, zero_shapes = [], [], [], []
    for alloc in nc.m.functions[0].allocations:
        if not isinstance(alloc, mybir.MemoryLocationSet):
            continue
        name = alloc.memorylocations[0].name
        if alloc.kind == "ExternalInput":
            if name != partition_name:
                in_names.append(name)
        elif alloc.kind == "ExternalOutput":
            shape = tuple(alloc.tensor_shape)
            dtype = mybir.dt.np(alloc.dtype)
            out_names.append(name)
            out_avals.append(jax.core.ShapedArray(shape, dtype))
            zero_shapes.append((shape, dtype))
    n_params = len(in_names)
    n_outs = len(out_avals)
    in_names_all = list(in_names) + list(out_names)
    if partition_name is not None:
        in_names_all.append(partition_name)
    donate = tuple(range(n_params, n_params + n_outs))

    def _body(*args):
        operands = list(args)
        if partition_name is not None:
            operands.append(b2j.partition_id_tensor())
        outs = b2j._bass_exec_p.bind(
            *operands,
            out_avals=tuple(out_avals),
            in_names=tuple(in_names_all),
            out_names=tuple(out_names),
            lowering_input_output_aliases=(),
            sim_require_finite=True,
            sim_require_nnan=True,
            nc=nc,
        )
        return tuple(outs)

    devices = jax.devices()[:n_cores]
    mesh = Mesh(np.asarray(devices), ("core",))
    gsharding = NamedSharding(mesh, PartitionSpec("core"))
    in_specs = (PartitionSpec("core"),) * (n_params + n_outs)
    out_specs = (PartitionSpec("core"),) * len(out_names)
    sharded = jax.jit(
        shard_map(_body, mesh=mesh, in_specs=in_specs, out_specs=out_specs,
                  check_rep=False),
        donate_argnums=donate, keep_unused=True)

    make_zeros = jax.jit(
        lambda: tuple(
            jnp.zeros((shape[0] * n_cores,) + tuple(shape[1:]), dtype)
            for shape, dtype in zero_shapes),
        out_shardings=(gsharding,) * n_outs)

    const_cache = {}

    def run(in_maps, volatile=("x",)):
        """in_maps: per-core dicts.  Inputs not named in `volatile` are
        fingerprinted by object id and kept device-resident across calls."""
        vol = [n for n in in_names if n in volatile]
        con = [n for n in in_names if n not in volatile]
        ckey = tuple(id(m[n]) for m in in_maps for n in con)
        if ckey not in const_cache:
            const_cache.clear()
            const_cache[ckey] = {
                n: jax.device_put(
                    np.concatenate([np.asarray(m[n]) for m in in_maps],
                                   axis=0), gsharding)
                for n in con}
        cdict = const_cache[ckey]
        gins = []
        for n in in_names:
            if n in cdict:
                gins.append(cdict[n])
            else:
                gins.append(jax.device_put(
                    np.concatenate([np.asarray(m[n]) for m in in_maps],
                                   axis=0), gsharding))
        gzeros = make_zeros()
        outs = sharded(*gins, *gzeros)
        res = [{} for _ in range(n_cores)]
        for i, name in enumerate(out_names):
            arr = np.asarray(outs[i])
            per = arr.shape[0] // n_cores
            for c in range(n_cores):
                res[c][name] = arr[c * per:(c + 1) * per]
        return res

    _RUNNER_CACHE[key] = run
    return run


_W_CACHE = {}


def _weights_cached(wqkv_w, wqkv_b, wo_w):
    key = (id(wqkv_w), id(wo_w), wqkv_w.shape, float(wqkv_w[0, 0]),
           float(wo_w[0, 0]), float(wqkv_w[-1, -1]), float(wo_w[-1, -1]))
    if key not in _W_CACHE:
        _W_CACHE.clear()
        _W_CACHE[key] = {hg: _host_weights(wqkv_w, wqkv_b, wo_w, hg)
                         for hg in (0, 1)}
    return _W_CACHE[key]


def kernel(layer_idx=None, inputs=None, wqkv_w=None, wqkv_b=None,
           wo_w=None, wo_b=None):
    inputs = np.asarray(inputs, dtype=np.float32)
    wqkv_w = np.asarray(wqkv_w, dtype=np.float32)
    wqkv_b = np.asarray(wqkv_b, dtype=np.float32)
    wo_w = np.asarray(wo_w, dtype=np.float32)
    wo_b = np.asarray(wo_b, dtype=np.float32)
    assert not np.any(wqkv_b), "nonzero wqkv_b not supported by this kernel build"

    x = np.ascontiguousarray(inputs.reshape(B * V, S, E))
    nc = _get_nc()
    wcache = _weights_cached(wqkv_w, wqkv_b, wo_w)

    in_maps = []
    for c in range(NCORE):
        g, hg = c // 2, c % 2
        wd = wcache[hg]
        m = dict(
            x=x[g],
            wqT=wd["wqT"], wkT=wd["wkT"], wvT=wd["wvT"], woT=wd["woT"],
            cs=wd["cs"], snS=wd["snS"], tri01d=wd["tri01d"],
            iden=wd["iden"], vones=wd["vones"], ones64=wd["ones64"],
        )
        in_maps.append(m)

    run = _get_runner(nc, NCORE)
    outs = run(in_maps)
    y = np.empty((B, V, S, E), dtype=np.float32)
    for g in range(B * V):
        np.add(outs[2 * g]["out"], outs[2 * g + 1]["out"], out=y[0, g])
    y += wo_b[None, None, None, :]
    return y
